# revision 28
# baseline (speedup 1.0000x reference)
"""Trainium2 Bass kernel for DownsampleConv2dGT (segment-mean pool -> 2x conv3x3 -> proj).

Contract: kernel(**inputs) takes FULL unsharded inputs (as from setup_inputs())
and returns the FULL output tuple (out [B, S, 512] f32, out_len [B] i32).

Sharding: pure data parallel — batch 16 -> 8 cores x 2 utterances.

Per-core device pipeline (per batch, split into 2 "units" of 2*Sc rows; each
unit puts its two Sc-row blocks on partition halves 0-63 / 64-127):
  1. pooling: pooled.T[d, s] = sum_t x[t, d] * Wt[t, s]   (PE, K=T chunks)
  2. conv1 (1->64ch, 3x3 SAME): im2col rows [18, L1] (9 shifted copies per
     block via DMA), one K=18 matmul per 512-col chunk with block-diag w1.
  3. conv2 (64->64ch): 9 accumulating matmuls per chunk with block-diag
     diag(W2_o, W2_o) [128,128]; rhs = shifted slices of padded h1. ReLU-copy
     into "dup" buffers; one extra shifted DMA per chunk creates the
     d-pair-stacked layout needed by the projection.
  4. proj: feature chunk k = wp[128k:128k+128] (contiguous, = d-pair x 64ch);
     lhsT = strided dup slice [128, Sc], rhs = streamed wp chunk [128, 512];
     accumulate 40 chunks in PSUM, add bias, DMA out.
"""

import math

import numpy as np

# ---- problem geometry (fixed by the nn.Module) ----
B, T, D = 16, 2048, 80
HID, OUT = 64, 512
NB = 64
DOWNSAMPLE = 4.0
DP = D + 2  # padded image row length (1 zero col each side)
NCORES = 8
KCH = T // 128  # 16 contraction chunks for pooling


# ---------------------------------------------------------------- host side
def _build_segments(x_len, boundaries, downsample=DOWNSAMPLE):
    """Identical to the reference's host-side segment construction."""
    segs_all = []
    for b in range(len(x_len)):
        L = int(x_len[b])
        bd = [int(v) for v in boundaries[b]]
        rate = L / len(bd) / downsample if downsample > 1.0 else 1.0
        if bd[-1] < L:
            bd.append(L)
        segs = []
        prev_t = 0
        for t in bd:
            t = min(t, L)
            if prev_t == t:
                continue
            if downsample > 1.0:
                delta_t = int(round((t - prev_t) / rate))
                if delta_t <= 0:
                    segs.append((prev_t, t))
                else:
                    _t = prev_t
                    while _t < t:
                        segs.append((_t, min(_t + delta_t, t)))
                        _t += delta_t
            else:
                segs.append((prev_t, t))
            prev_t = t
        segs_all.append(segs)
    return segs_all


def _make_wt(segs, S4):
    """Transposed, halo-padded pooling matrix [T, S4]; col 2+i = segment i."""
    wt = np.zeros((T, S4), np.float32)
    for i, (s, e) in enumerate(segs):
        wt[s:e, 2 + i] = 1.0 / (e - s)
    return wt


# ---------------------------------------------------------------- bass build
_CACHE = {}


def _build_program(Sc, S, zb):
    import concourse.bass as bass
    import concourse.mybir as mybir
    import concourse.tile as tile
    from concourse import bacc

    f32 = mybir.dt.float32
    f16 = mybir.dt.float16
    Sk = 4 * Sc
    S4 = Sk + 4
    NP4 = Sc + 4          # pooled cols per block (2-halo each side)
    L1 = (Sc + 2) * DP    # h1 interior length (conv1 output rows = Sc+2)
    L2 = Sc * DP          # conv2 output length per block
    N1C = math.ceil(L1 / 512)
    N2C = math.ceil(L2 / 512)
    NKP = (HID * D) // 128  # 40 projection K-chunks

    nc = bacc.Bacc("TRN2", target_bir_lowering=False, debug=False, num_devices=NCORES)

    x_d = nc.dram_tensor("x2", [2, T, D], f16, kind="ExternalInput")
    wt_d = nc.dram_tensor("wtp", [2, T, S4], f16, kind="ExternalInput")
    w1_d = nc.dram_tensor("w1bd", [18, 128], f16, kind="ExternalInput")
    w2_d = nc.dram_tensor("w2bd", [9, 128, 128], f16, kind="ExternalInput")
    wp_d = nc.dram_tensor("wpc", [NKP, 128, OUT], f16, kind="ExternalInput")
    bp_d = nc.dram_tensor("bp2", [1, OUT], f32, kind="ExternalInput")
    b1_d = nc.dram_tensor("b1c", [128, 1], f32, kind="ExternalInput")
    b2_d = nc.dram_tensor("b2c", [128, 1], f32, kind="ExternalInput")
    out_d = nc.dram_tensor("out2", [2, Sk, OUT], f32, kind="ExternalOutput")

    Relu = mybir.ActivationFunctionType.Relu
    Copy = mybir.ActivationFunctionType.Copy

    with tile.TileContext(nc) as tc:
        with (
            tc.tile_pool(name="consts", bufs=1) as cpool,
            tc.tile_pool(name="xp", bufs=2) as xpool,
            tc.tile_pool(name="wtp", bufs=2) as wtpool,
            tc.tile_pool(name="pooled", bufs=4) as plpool,
            tc.tile_pool(name="rhs18", bufs=2) as rpool,
            tc.tile_pool(name="h1", bufs=2) as h1pool,
            tc.tile_pool(name="dup0", bufs=1) as d0pool,
            tc.tile_pool(name="dup1", bufs=1) as d1pool,
            tc.tile_pool(name="outp", bufs=3) as opool,
            tc.tile_pool(name="pps", bufs=2, space=bass.MemorySpace.PSUM) as pl_ps,
            tc.tile_pool(name="cps", bufs=4, space=bass.MemorySpace.PSUM) as c_ps,
            tc.tile_pool(name="jps", bufs=2, space=bass.MemorySpace.PSUM) as j_ps,
        ):
            w1_sb = cpool.tile([18, 128], f16)
            nc.sync.dma_start(out=w1_sb[:], in_=w1_d[:])
            w2_sb = cpool.tile([128, 9, 128], f16)
            nc.sync.dma_start(out=w2_sb[:], in_=w2_d[:].rearrange("o k m -> k o m"))
            bp_sb = cpool.tile([1, OUT], f32)
            nc.sync.dma_start(out=bp_sb[:], in_=bp_d[:])
            bp_bc = cpool.tile([128, OUT], f32)
            nc.gpsimd.partition_broadcast(bp_bc[:], bp_sb[0:1, :])
            b1_sb = cpool.tile([128, 1], f32)
            nc.sync.dma_start(out=b1_sb[:], in_=b1_d[:])
            b2_sb = cpool.tile([128, 1], f32)
            nc.sync.dma_start(out=b2_sb[:], in_=b2_d[:])
            wp_sb = cpool.tile([128, NKP, OUT], f16)
            nc.sync.dma_start(out=wp_sb[:], in_=wp_d[:].rearrange("k p o -> p k o"))

            xsb_of = {}

            def pool_im2col(b, u):
                """Pooling + im2col for one unit; returns the r18 tile.

                Issued one unit ahead so the slow strided im2col DMAs hide
                under the previous unit's conv2+proj PE work.
                """
                if b not in xsb_of:
                    x_sb = xpool.tile([128, KCH, D], f16, tag="xsb", name=f"xsb{b}")
                    xr = x_d[b].rearrange("(k p) d -> p k d", p=128)
                    nc.sync.dma_start(out=x_sb[:, 0:4, :], in_=xr[:, 0:4, :])
                    nc.sync.dma_start(out=x_sb[:, 4:KCH, :], in_=xr[:, 4:KCH, :])
                    xsb_of[b] = x_sb
                x_sb = xsb_of[b]
                pooled = []
                for h in range(2):
                    s0 = (2 * u + h) * Sc
                    wt_sb = wtpool.tile([128, KCH, NP4], f16, tag="wtsb")
                    wtr = wt_d[b, :, s0 : s0 + NP4].rearrange(
                        "(k p) s -> p k s", p=128
                    )
                    nc.sync.dma_start(out=wt_sb[:, 0:4, :], in_=wtr[:, 0:4, :])
                    nc.sync.dma_start(out=wt_sb[:, 4:KCH, :], in_=wtr[:, 4:KCH, :])
                    pp = pl_ps.tile([NP4, D], f32, tag="plps")
                    for k in range(KCH):
                        nc.tensor.matmul(
                            pp[:],
                            wt_sb[:, k, :],
                            x_sb[:, k, :],
                            start=(k == 0),
                            stop=(k == KCH - 1),
                        )
                    po = plpool.tile([NP4, D], f16, tag="pooled")
                    nc.scalar.activation(po[:], pp[:], Copy)
                    pooled.append(po)

                r18 = rpool.tile([18, Sc + 2, DP], f16, tag="r18")
                nc.vector.memset(r18[:, :, 0:1], 0.0)
                nc.vector.memset(r18[:, :, D - 1 : D + 2], 0.0)
                for h in range(2):
                    for dy in range(3):
                        for dx in range(3):
                            r = 9 * h + 3 * dy + dx
                            if dx == 0:
                                d0, sp, cnt = 1, 0, D - 1
                            elif dx == 1:
                                d0, sp, cnt = 0, 0, D
                            else:
                                d0, sp, cnt = 0, 1, D - 1
                            nc.sync.dma_start(
                                out=r18[r : r + 1, :, d0 : d0 + cnt],
                                in_=pooled[h][dy : dy + Sc + 2, sp : sp + cnt],
                            )
                return r18

            units = [(b, u) for b in range(2) for u in range(2)]
            r18_cur = pool_im2col(*units[0])
            for idx, (b, u) in enumerate(units):
                    r18f = r18_cur.rearrange("p u d -> p (u d)")

                    # ---------------- conv1: K=18 matmuls per 512 chunk
                    h1 = h1pool.tile([128, 1 + L1 + 2], f16, tag="h1")
                    nc.vector.memset(h1[:, 1 + L1 : 1 + L1 + 2], 0.0)
                    for c in range(N1C):
                        n = min(512, L1 - 512 * c)
                        ps = c_ps.tile([128, 512], f32, tag="cps")
                        nc.tensor.matmul(
                            ps[:, :n],
                            w1_sb[:],
                            r18f[:, 512 * c : 512 * c + n],
                            start=True,
                            stop=True,
                        )
                        if zb and c % 2 == 1:
                            nc.vector.tensor_relu(
                                h1[:, 1 + 512 * c : 1 + 512 * c + n], ps[:, :n]
                            )
                        else:
                            nc.scalar.activation(
                                h1[:, 1 + 512 * c : 1 + 512 * c + n],
                                ps[:, :n],
                                Relu,
                                bias=b1_sb[:],
                            )
                    nc.vector.memset(h1[:, 0:1], 0.0)
                    h1r = h1[:, 1 : 1 + L1].rearrange("p (u d) -> p u d", d=DP)
                    nrows = Sc + 2
                    for q in range(4):
                        r0 = q * nrows // 4
                        r1 = (q + 1) * nrows // 4
                        nc.vector.memset(h1r[:, r0:r1, D : D + 2], 0.0)
                    # conv2's input must be zero outside global rows [0, S):
                    # row u of block h maps to s = (2u_blk+h)*Sc - 1 + u.
                    for h in range(2):
                        s0 = (2 * u + h) * Sc
                        pp0, pp1 = (0, 64) if h == 0 else (64, 128)
                        if s0 == 0:
                            nc.vector.memset(h1[pp0:pp1, 1 : 1 + DP], 0.0)
                        zs = max(0, S - s0 + 1)
                        if zs <= Sc + 1:
                            nc.vector.memset(
                                h1[pp0:pp1, 1 + zs * DP : 1 + (Sc + 2) * DP], 0.0
                            )

                    # prefetch next unit's pooling + im2col
                    if idx + 1 < len(units):
                        r18_next = pool_im2col(*units[idx + 1])
                    else:
                        r18_next = None

                    # ---------------- conv2: 9 accumulating matmuls per chunk
                    dup0 = d0pool.tile([128, L2 + 2], f16, tag="dup0")
                    dup1 = d1pool.tile([128, L2 + 2], f16, tag="dup1")
                    nc.vector.memset(dup0[:, L2 : L2 + 2], 0.0)
                    nc.vector.memset(dup1[:, 0:1], 0.0)
                    nc.vector.memset(dup1[:, L2 : L2 + 2], 0.0)
                    for c in range(N2C):
                        n = min(512, L2 - 512 * c)
                        ps = c_ps.tile([128, 512], f32, tag="cps")
                        for o in range(9):
                            off = (o // 3) * DP + (o % 3) - 1
                            nc.tensor.matmul(
                                ps[:, :n],
                                w2_sb[:, o, :],
                                h1[:, 1 + 512 * c + off : 1 + 512 * c + off + n],
                                start=(o == 0),
                                stop=(o == 8),
                            )
                        nc.scalar.activation(
                            dup0[0:64, 512 * c : 512 * c + n],
                            ps[0:64, :n],
                            Relu,
                            bias=b2_sb[0:64, :],
                        )
                        if zb:
                            nc.vector.tensor_relu(
                                dup1[64:128, 512 * c : 512 * c + n], ps[64:128, :n]
                            )
                        else:
                            nc.scalar.activation(
                                dup1[64:128, 512 * c : 512 * c + n],
                                ps[64:128, :n],
                                Relu,
                                bias=b2_sb[64:128, :],
                            )
                    # d-pair replication (cross-partition, one shifted DMA per chunk)
                    for c in range(N2C):
                        n = min(512, L2 - 512 * c)
                        nc.sync.dma_start(
                            out=dup0[64:128, 512 * c : 512 * c + n],
                            in_=dup0[0:64, 512 * c + 1 : 512 * c + 1 + n],
                        )
                        nc.sync.dma_start(
                            out=dup1[0:64, 512 * c + 1 : 512 * c + 1 + n],
                            in_=dup1[64:128, 512 * c : 512 * c + n],
                        )
                    dup0r = dup0[:, 0:L2].rearrange("p (u d) -> p d u", d=DP)
                    dup1r = dup1[:, 0:L2].rearrange("p (u d) -> p d u", d=DP)

                    # ---------------- projection: 40 K-chunks, wp streamed
                    pj0 = j_ps.tile([128, OUT], f32, tag="jps", name="pj0")
                    pj1 = j_ps.tile([128, OUT], f32, tag="jps", name="pj1")
                    pj = [pj0, pj1]
                    for k in range(NKP):
                        nc.tensor.matmul(
                            pj[0][:Sc, :],
                            dup0r[:, 2 * k, :],
                            wp_sb[:, k, :],
                            start=(k == 0),
                            stop=(k == NKP - 1),
                        )
                        nc.tensor.matmul(
                            pj[1][:Sc, :],
                            dup1r[:, 2 * k + 1, :],
                            wp_sb[:, k, :],
                            start=(k == 0),
                            stop=(k == NKP - 1),
                        )
                    for h in range(2):
                        s0 = (2 * u + h) * Sc
                        ob = opool.tile([128, OUT], f32, tag="outsb")
                        nc.vector.tensor_add(
                            ob[:Sc, :], pj[h][:Sc, :], bp_bc[:Sc, :]
                        )
                        nc.sync.dma_start(
                            out=out_d[b, s0 : s0 + Sc, :], in_=ob[:Sc, :]
                        )
                    r18_cur = r18_next

    nc.compile()
    return nc


def _get_program(Sc, S, zb):
    key = (Sc, S, zb)
    if key not in _CACHE:
        _CACHE[key] = _build_program(Sc, S, zb)
    return _CACHE[key]


def _prep_inputs(x, x_len, boundaries, w1, w2, wp, bp):
    """Host prep: segments, pooling matrices, block-diag weights, per-core maps."""
    x = np.asarray(x, np.float32)
    x_len = np.asarray(x_len)
    boundaries = np.asarray(boundaries)
    w1 = np.asarray(w1, np.float32)
    w2 = np.asarray(w2, np.float32)
    wp = np.asarray(wp, np.float32)
    bp = np.asarray(bp, np.float32)

    segs_all = _build_segments(x_len, boundaries)
    out_len = np.array([len(s) for s in segs_all], np.int32)
    S = int(out_len.max())
    Sc = math.ceil(S / 4)
    Sk = 4 * Sc
    S4 = Sk + 4
    NKP = (HID * D) // 128

    w1bd = np.zeros((18, 128), np.float16)
    w1r = w1.reshape(HID, 9)  # [c, (dy,dx)]
    w1bd[0:9, 0:64] = w1r.T
    w1bd[9:18, 64:128] = w1r.T

    w2bd = np.zeros((9, 128, 128), np.float16)
    for o in range(9):
        blk = w2[:, :, o // 3, o % 3].T  # [c_in, c_out]
        w2bd[o, 0:64, 0:64] = blk
        w2bd[o, 64:128, 64:128] = blk

    wpc = np.ascontiguousarray(wp.reshape(NKP, 128, OUT).astype(np.float16))
    bp2 = bp.reshape(1, OUT)

    in_maps = []
    for c in range(NCORES):
        bb = [2 * c, 2 * c + 1]
        wtp = np.stack([_make_wt(segs_all[i], S4) for i in bb])
        in_maps.append(
            {
                "x2": np.ascontiguousarray(x[bb]).astype(np.float16),
                "wtp": wtp.astype(np.float16),
                "w1bd": w1bd,
                "w2bd": w2bd,
                "wpc": wpc,
                "bp2": bp2,
            }
        )
    return in_maps, out_len, S, Sc


def kernel(x, x_len, boundaries, w1, b1, w2, b2, wp, bp):
    from concourse.bass_utils import run_bass_kernel_spmd

    in_maps, out_len, S, Sc = _prep_inputs(x, x_len, boundaries, w1, w2, wp, bp)
    b1c = np.tile(np.asarray(b1, np.float32), 2).reshape(128, 1)
    b2c = np.tile(np.asarray(b2, np.float32), 2).reshape(128, 1)
    for m in in_maps:
        m["b1c"] = b1c
        m["b2c"] = b2c

    zb = bool(not np.any(b1c) and not np.any(b2c))
    nc = _get_program(Sc, S, zb)
    res = run_bass_kernel_spmd(nc, in_maps, list(range(NCORES)))
    out = np.empty((B, S, OUT), np.float32)
    for c in range(NCORES):
        out[2 * c : 2 * c + 2] = res.results[c]["out2"][:, :S, :]
    return out, out_len


# revision 29
# speedup vs baseline: 1.0227x; 1.0227x over previous
"""Trainium2 Bass kernel for DownsampleConv2dGT (segment-mean pool -> 2x conv3x3 -> proj).

Contract: kernel(**inputs) takes FULL unsharded inputs (as from setup_inputs())
and returns the FULL output tuple (out [B, S, 512] f32, out_len [B] i32).

Sharding: pure data parallel — batch 16 -> 8 cores x 2 utterances.

Per-core device pipeline (per batch, split into 2 "units" of 2*Sc rows; each
unit puts its two Sc-row blocks on partition halves 0-63 / 64-127):
  1. pooling: pooled.T[d, s] = sum_t x[t, d] * Wt[t, s]   (PE, K=T chunks)
  2. conv1 (1->64ch, 3x3 SAME): im2col rows [18, L1] (9 shifted copies per
     block via DMA), one K=18 matmul per 512-col chunk with block-diag w1.
  3. conv2 (64->64ch): 9 accumulating matmuls per chunk with block-diag
     diag(W2_o, W2_o) [128,128]; rhs = shifted slices of padded h1. ReLU-copy
     into "dup" buffers; one extra shifted DMA per chunk creates the
     d-pair-stacked layout needed by the projection.
  4. proj: feature chunk k = wp[128k:128k+128] (contiguous, = d-pair x 64ch);
     lhsT = strided dup slice [128, Sc], rhs = streamed wp chunk [128, 512];
     accumulate 40 chunks in PSUM, add bias, DMA out.
"""

import math

import numpy as np

# ---- problem geometry (fixed by the nn.Module) ----
B, T, D = 16, 2048, 80
HID, OUT = 64, 512
NB = 64
DOWNSAMPLE = 4.0
DP = D + 2  # padded image row length (1 zero col each side)
NCORES = 8
KCH = T // 128  # 16 contraction chunks for pooling


# ---------------------------------------------------------------- host side
def _build_segments(x_len, boundaries, downsample=DOWNSAMPLE):
    """Identical to the reference's host-side segment construction."""
    segs_all = []
    for b in range(len(x_len)):
        L = int(x_len[b])
        bd = [int(v) for v in boundaries[b]]
        rate = L / len(bd) / downsample if downsample > 1.0 else 1.0
        if bd[-1] < L:
            bd.append(L)
        segs = []
        prev_t = 0
        for t in bd:
            t = min(t, L)
            if prev_t == t:
                continue
            if downsample > 1.0:
                delta_t = int(round((t - prev_t) / rate))
                if delta_t <= 0:
                    segs.append((prev_t, t))
                else:
                    _t = prev_t
                    while _t < t:
                        segs.append((_t, min(_t + delta_t, t)))
                        _t += delta_t
            else:
                segs.append((prev_t, t))
            prev_t = t
        segs_all.append(segs)
    return segs_all


def _make_wt(segs, S4):
    """Transposed, halo-padded pooling matrix [T, S4]; col 2+i = segment i."""
    wt = np.zeros((T, S4), np.float32)
    for i, (s, e) in enumerate(segs):
        wt[s:e, 2 + i] = 1.0 / (e - s)
    return wt


# ---------------------------------------------------------------- bass build
_CACHE = {}


def _build_program(Sc, S, zb):
    import concourse.bass as bass
    import concourse.mybir as mybir
    import concourse.tile as tile
    from concourse import bacc

    f32 = mybir.dt.float32
    f16 = mybir.dt.float16
    Sk = 4 * Sc
    S4 = Sk + 4
    NP4 = Sc + 4          # pooled cols per block (2-halo each side)
    L1 = (Sc + 2) * DP    # h1 interior length (conv1 output rows = Sc+2)
    L2 = Sc * DP          # conv2 output length per block
    N1C = math.ceil(L1 / 512)
    N2C = math.ceil(L2 / 512)
    NKP = (HID * D) // 128  # 40 projection K-chunks

    nc = bacc.Bacc("TRN2", target_bir_lowering=False, debug=False, num_devices=NCORES)

    x_d = nc.dram_tensor("x2", [2, T, D], f16, kind="ExternalInput")
    wt_d = nc.dram_tensor("wtp", [2, T, S4], f16, kind="ExternalInput")
    w1_d = nc.dram_tensor("w1bd", [18, 128], f16, kind="ExternalInput")
    w2_d = nc.dram_tensor("w2bd", [9, 128, 128], f16, kind="ExternalInput")
    wp_d = nc.dram_tensor("wpc", [NKP, 128, OUT], f16, kind="ExternalInput")
    bp_d = nc.dram_tensor("bp2", [1, OUT], f32, kind="ExternalInput")
    b1_d = nc.dram_tensor("b1c", [128, 1], f32, kind="ExternalInput")
    b2_d = nc.dram_tensor("b2c", [128, 1], f32, kind="ExternalInput")
    out_d = nc.dram_tensor("out2", [2, Sk, OUT], f32, kind="ExternalOutput")

    Relu = mybir.ActivationFunctionType.Relu
    Copy = mybir.ActivationFunctionType.Copy

    with tile.TileContext(nc) as tc:
        with (
            tc.tile_pool(name="consts", bufs=1) as cpool,
            tc.tile_pool(name="xp", bufs=2) as xpool,
            tc.tile_pool(name="wtp", bufs=2) as wtpool,
            tc.tile_pool(name="pooled", bufs=4) as plpool,
            tc.tile_pool(name="rhs18", bufs=2) as rpool,
            tc.tile_pool(name="h1", bufs=2) as h1pool,
            tc.tile_pool(name="dup0", bufs=1) as d0pool,
            tc.tile_pool(name="dup1", bufs=1) as d1pool,
            tc.tile_pool(name="outp", bufs=3) as opool,
            tc.tile_pool(name="pps", bufs=2, space=bass.MemorySpace.PSUM) as pl_ps,
            tc.tile_pool(name="cps", bufs=4, space=bass.MemorySpace.PSUM) as c_ps,
            tc.tile_pool(name="jps", bufs=2, space=bass.MemorySpace.PSUM) as j_ps,
        ):
            w1_sb = cpool.tile([18, 128], f16)
            nc.sync.dma_start(out=w1_sb[:], in_=w1_d[:])
            w2_sb = cpool.tile([128, 9, 128], f16)
            nc.sync.dma_start(out=w2_sb[:], in_=w2_d[:].rearrange("o k m -> k o m"))
            bp_sb = cpool.tile([1, OUT], f32)
            nc.sync.dma_start(out=bp_sb[:], in_=bp_d[:])
            bp_bc = cpool.tile([128, OUT], f32)
            nc.gpsimd.partition_broadcast(bp_bc[:], bp_sb[0:1, :])
            b1_sb = cpool.tile([128, 1], f32)
            nc.sync.dma_start(out=b1_sb[:], in_=b1_d[:])
            b2_sb = cpool.tile([128, 1], f32)
            nc.sync.dma_start(out=b2_sb[:], in_=b2_d[:])
            wp_sb = cpool.tile([128, NKP, OUT], f16)
            nc.sync.dma_start(out=wp_sb[:], in_=wp_d[:].rearrange("k p o -> p k o"))

            # HAM warmup: the first ~87us otherwise run at K=4/8 (1.2GHz)
            # because startup DMA waits keep re-throttling the PE. Scratch
            # matmuls fill those idle windows and hold the clock at 2.4GHz.
            wu = cpool.tile([128, 512], f16)
            nc.vector.memset(wu[:], 0.0)

            def warmup(n, tag):
                for i in range(n):
                    wps = c_ps.tile([128, 512], f32, tag="cps", name=f"wu{tag}_{i}")
                    nc.tensor.matmul(
                        wps[:], wu[:, 0:128], wu[:], start=True, stop=True
                    )

            warmup(24, "a")

            xsb_of = {}

            def pool_im2col(b, u):
                """Pooling + im2col for one unit; returns the r18 tile.

                Issued one unit ahead so the slow strided im2col DMAs hide
                under the previous unit's conv2+proj PE work.
                """
                if b not in xsb_of:
                    x_sb = xpool.tile([128, KCH, D], f16, tag="xsb", name=f"xsb{b}")
                    xr = x_d[b].rearrange("(k p) d -> p k d", p=128)
                    nc.sync.dma_start(out=x_sb[:, 0:4, :], in_=xr[:, 0:4, :])
                    nc.sync.dma_start(out=x_sb[:, 4:KCH, :], in_=xr[:, 4:KCH, :])
                    xsb_of[b] = x_sb
                x_sb = xsb_of[b]
                pooled = []
                for h in range(2):
                    s0 = (2 * u + h) * Sc
                    wt_sb = wtpool.tile([128, KCH, NP4], f16, tag="wtsb")
                    wtr = wt_d[b, :, s0 : s0 + NP4].rearrange(
                        "(k p) s -> p k s", p=128
                    )
                    nc.sync.dma_start(out=wt_sb[:, 0:4, :], in_=wtr[:, 0:4, :])
                    nc.sync.dma_start(out=wt_sb[:, 4:KCH, :], in_=wtr[:, 4:KCH, :])
                    pp = pl_ps.tile([NP4, D], f32, tag="plps")
                    for k in range(KCH):
                        nc.tensor.matmul(
                            pp[:],
                            wt_sb[:, k, :],
                            x_sb[:, k, :],
                            start=(k == 0),
                            stop=(k == KCH - 1),
                        )
                    po = plpool.tile([NP4, D], f16, tag="pooled")
                    nc.scalar.activation(po[:], pp[:], Copy)
                    pooled.append(po)

                r18 = rpool.tile([18, Sc + 2, DP], f16, tag="r18")
                nc.vector.memset(r18[:, :, 0:1], 0.0)
                nc.vector.memset(r18[:, :, D - 1 : D + 2], 0.0)
                for h in range(2):
                    for dy in range(3):
                        for dx in range(3):
                            r = 9 * h + 3 * dy + dx
                            if dx == 0:
                                d0, sp, cnt = 1, 0, D - 1
                            elif dx == 1:
                                d0, sp, cnt = 0, 0, D
                            else:
                                d0, sp, cnt = 0, 1, D - 1
                            nc.sync.dma_start(
                                out=r18[r : r + 1, :, d0 : d0 + cnt],
                                in_=pooled[h][dy : dy + Sc + 2, sp : sp + cnt],
                            )
                return r18

            units = [(b, u) for b in range(2) for u in range(2)]
            r18_cur = pool_im2col(*units[0])
            warmup(48, "b")
            for idx, (b, u) in enumerate(units):
                    r18f = r18_cur.rearrange("p u d -> p (u d)")

                    # ---------------- conv1: K=18 matmuls per 512 chunk
                    h1 = h1pool.tile([128, 1 + L1 + 2], f16, tag="h1")
                    nc.vector.memset(h1[:, 1 + L1 : 1 + L1 + 2], 0.0)
                    for c in range(N1C):
                        n = min(512, L1 - 512 * c)
                        ps = c_ps.tile([128, 512], f32, tag="cps")
                        nc.tensor.matmul(
                            ps[:, :n],
                            w1_sb[:],
                            r18f[:, 512 * c : 512 * c + n],
                            start=True,
                            stop=True,
                        )
                        if zb and c % 2 == 1:
                            nc.vector.tensor_relu(
                                h1[:, 1 + 512 * c : 1 + 512 * c + n], ps[:, :n]
                            )
                        else:
                            nc.scalar.activation(
                                h1[:, 1 + 512 * c : 1 + 512 * c + n],
                                ps[:, :n],
                                Relu,
                                bias=b1_sb[:],
                            )
                    nc.vector.memset(h1[:, 0:1], 0.0)
                    h1r = h1[:, 1 : 1 + L1].rearrange("p (u d) -> p u d", d=DP)
                    nrows = Sc + 2
                    for q in range(4):
                        r0 = q * nrows // 4
                        r1 = (q + 1) * nrows // 4
                        nc.vector.memset(h1r[:, r0:r1, D : D + 2], 0.0)
                    # conv2's input must be zero outside global rows [0, S):
                    # row u of block h maps to s = (2u_blk+h)*Sc - 1 + u.
                    for h in range(2):
                        s0 = (2 * u + h) * Sc
                        pp0, pp1 = (0, 64) if h == 0 else (64, 128)
                        if s0 == 0:
                            nc.vector.memset(h1[pp0:pp1, 1 : 1 + DP], 0.0)
                        zs = max(0, S - s0 + 1)
                        if zs <= Sc + 1:
                            nc.vector.memset(
                                h1[pp0:pp1, 1 + zs * DP : 1 + (Sc + 2) * DP], 0.0
                            )

                    # prefetch next unit's pooling + im2col
                    if idx + 1 < len(units):
                        r18_next = pool_im2col(*units[idx + 1])
                    else:
                        r18_next = None

                    # ---------------- conv2: 9 accumulating matmuls per chunk
                    dup0 = d0pool.tile([128, L2 + 2], f16, tag="dup0")
                    dup1 = d1pool.tile([128, L2 + 2], f16, tag="dup1")
                    nc.vector.memset(dup0[:, L2 : L2 + 2], 0.0)
                    nc.vector.memset(dup1[:, 0:1], 0.0)
                    nc.vector.memset(dup1[:, L2 : L2 + 2], 0.0)
                    for c in range(N2C):
                        n = min(512, L2 - 512 * c)
                        ps = c_ps.tile([128, 512], f32, tag="cps")
                        for o in range(9):
                            off = (o // 3) * DP + (o % 3) - 1
                            nc.tensor.matmul(
                                ps[:, :n],
                                w2_sb[:, o, :],
                                h1[:, 1 + 512 * c + off : 1 + 512 * c + off + n],
                                start=(o == 0),
                                stop=(o == 8),
                            )
                        nc.scalar.activation(
                            dup0[0:64, 512 * c : 512 * c + n],
                            ps[0:64, :n],
                            Relu,
                            bias=b2_sb[0:64, :],
                        )
                        if zb:
                            nc.vector.tensor_relu(
                                dup1[64:128, 512 * c : 512 * c + n], ps[64:128, :n]
                            )
                        else:
                            nc.scalar.activation(
                                dup1[64:128, 512 * c : 512 * c + n],
                                ps[64:128, :n],
                                Relu,
                                bias=b2_sb[64:128, :],
                            )
                    # d-pair replication (cross-partition, one shifted DMA per chunk)
                    for c in range(N2C):
                        n = min(512, L2 - 512 * c)
                        nc.sync.dma_start(
                            out=dup0[64:128, 512 * c : 512 * c + n],
                            in_=dup0[0:64, 512 * c + 1 : 512 * c + 1 + n],
                        )
                        nc.sync.dma_start(
                            out=dup1[0:64, 512 * c + 1 : 512 * c + 1 + n],
                            in_=dup1[64:128, 512 * c : 512 * c + n],
                        )
                    dup0r = dup0[:, 0:L2].rearrange("p (u d) -> p d u", d=DP)
                    dup1r = dup1[:, 0:L2].rearrange("p (u d) -> p d u", d=DP)

                    # ---------------- projection: 40 K-chunks, wp streamed
                    pj0 = j_ps.tile([128, OUT], f32, tag="jps", name="pj0")
                    pj1 = j_ps.tile([128, OUT], f32, tag="jps", name="pj1")
                    pj = [pj0, pj1]
                    for k in range(NKP):
                        nc.tensor.matmul(
                            pj[0][:Sc, :],
                            dup0r[:, 2 * k, :],
                            wp_sb[:, k, :],
                            start=(k == 0),
                            stop=(k == NKP - 1),
                        )
                        nc.tensor.matmul(
                            pj[1][:Sc, :],
                            dup1r[:, 2 * k + 1, :],
                            wp_sb[:, k, :],
                            start=(k == 0),
                            stop=(k == NKP - 1),
                        )
                    for h in range(2):
                        s0 = (2 * u + h) * Sc
                        ob = opool.tile([128, OUT], f32, tag="outsb")
                        nc.vector.tensor_add(
                            ob[:Sc, :], pj[h][:Sc, :], bp_bc[:Sc, :]
                        )
                        nc.sync.dma_start(
                            out=out_d[b, s0 : s0 + Sc, :], in_=ob[:Sc, :]
                        )
                    r18_cur = r18_next

    nc.compile()
    return nc


def _get_program(Sc, S, zb):
    key = (Sc, S, zb)
    if key not in _CACHE:
        _CACHE[key] = _build_program(Sc, S, zb)
    return _CACHE[key]


def _prep_inputs(x, x_len, boundaries, w1, w2, wp, bp):
    """Host prep: segments, pooling matrices, block-diag weights, per-core maps."""
    x = np.asarray(x, np.float32)
    x_len = np.asarray(x_len)
    boundaries = np.asarray(boundaries)
    w1 = np.asarray(w1, np.float32)
    w2 = np.asarray(w2, np.float32)
    wp = np.asarray(wp, np.float32)
    bp = np.asarray(bp, np.float32)

    segs_all = _build_segments(x_len, boundaries)
    out_len = np.array([len(s) for s in segs_all], np.int32)
    S = int(out_len.max())
    Sc = math.ceil(S / 4)
    Sk = 4 * Sc
    S4 = Sk + 4
    NKP = (HID * D) // 128

    w1bd = np.zeros((18, 128), np.float16)
    w1r = w1.reshape(HID, 9)  # [c, (dy,dx)]
    w1bd[0:9, 0:64] = w1r.T
    w1bd[9:18, 64:128] = w1r.T

    w2bd = np.zeros((9, 128, 128), np.float16)
    for o in range(9):
        blk = w2[:, :, o // 3, o % 3].T  # [c_in, c_out]
        w2bd[o, 0:64, 0:64] = blk
        w2bd[o, 64:128, 64:128] = blk

    wpc = np.ascontiguousarray(wp.reshape(NKP, 128, OUT).astype(np.float16))
    bp2 = bp.reshape(1, OUT)

    in_maps = []
    for c in range(NCORES):
        bb = [2 * c, 2 * c + 1]
        wtp = np.stack([_make_wt(segs_all[i], S4) for i in bb])
        in_maps.append(
            {
                "x2": np.ascontiguousarray(x[bb]).astype(np.float16),
                "wtp": wtp.astype(np.float16),
                "w1bd": w1bd,
                "w2bd": w2bd,
                "wpc": wpc,
                "bp2": bp2,
            }
        )
    return in_maps, out_len, S, Sc


def kernel(x, x_len, boundaries, w1, b1, w2, b2, wp, bp):
    from concourse.bass_utils import run_bass_kernel_spmd

    in_maps, out_len, S, Sc = _prep_inputs(x, x_len, boundaries, w1, w2, wp, bp)
    b1c = np.tile(np.asarray(b1, np.float32), 2).reshape(128, 1)
    b2c = np.tile(np.asarray(b2, np.float32), 2).reshape(128, 1)
    for m in in_maps:
        m["b1c"] = b1c
        m["b2c"] = b2c

    zb = bool(not np.any(b1c) and not np.any(b2c))
    nc = _get_program(Sc, S, zb)
    res = run_bass_kernel_spmd(nc, in_maps, list(range(NCORES)))
    out = np.empty((B, S, OUT), np.float32)
    for c in range(NCORES):
        out[2 * c : 2 * c + 2] = res.results[c]["out2"][:, :S, :]
    return out, out_len
